# revision 16
# baseline (speedup 1.0000x reference)
"""Trainium2 Bass kernel for an attention-augmented GRU cell (CGRUCell).

Reference computation (per batch row):
    cache   = context @ Wk.T + bk                  # [S, A]
    q       = hidden @ Wq.T + bq                   # [A]
    logits  = tanh(q + cache) @ Wl[0] + bl         # [S]
    logits  = where(mask, -1e18, logits)
    w       = softmax(logits)                      # [S]
    attn    = w @ context                          # [CTX]
    x       = input @ We.T + be + attn @ Wa.T + ba
    gx      = x @ W_ih.T + b_ih ; gh = hidden @ W_hh.T + b_hh
    r, z    = sigmoid(gx_r + gh_r), sigmoid(gx_z + gh_z)
    n       = tanh(gx_n + r * gh_n)
    hidden1 = (1 - z) * n + z * hidden
Outputs: (hidden1, attn)

Strategy: data-parallel over batch on 8 NeuronCores (8 rows each). The
dominant work is the [S,CTX]@[CTX,A] key projection; it runs on the
TensorEngine in bf16 (context is cast on the otherwise-idle GpSimd
engine, then transposed with cheap bf16 identity-matmuls so the
contraction dim lands on partitions). The softmax reduction over the
attention dim rides on f32r matmuls against Wl with the q/bk bias fused
into the tanh Activation op; the attention-value matvec contracts the
full-precision f32r context so the attn output keeps ~1e-4 accuracy.
The GRU algebra is reassociated (W1 = W_ih@We, W2 = W_ih@Wa) so all of
it except attn @ W2.T is computed in a prologue from the raw inputs.
The softmax/attention tail of each batch row is emitted interleaved
into the next row's cache matmuls so the in-order PE never idles on
DVE/ACT latency.
"""

import sys

if "/opt/trn_rl_repo" not in sys.path:
    sys.path.insert(0, "/opt/trn_rl_repo")

import ml_dtypes
import numpy as np

import concourse.bass as bass
import concourse.tile as tile
from concourse import bacc, mybir
from concourse.bass_utils import run_bass_kernel_spmd

NCORES = 8
B, S, IN, HID, CTX, ATT = 64, 1024, 1024, 1024, 1024, 1024
BL = B // NCORES          # batch rows per core
H3 = 3 * HID
F32 = mybir.dt.float32
F32R = mybir.dt.float32r
BF16 = mybir.dt.bfloat16
AX = mybir.AxisListType
AF = mybir.ActivationFunctionType
BF16NP = ml_dtypes.bfloat16


def _r(ap):
    return ap.bitcast(F32R)


def build_program():
    nc = bacc.Bacc("TRN2", target_bir_lowering=False, debug=False, num_devices=NCORES)

    d_ctx = nc.dram_tensor("ctx", [BL, S, CTX], F32R, kind="ExternalInput").ap()
    d_pen = nc.dram_tensor("pen", [BL, S], F32R, kind="ExternalInput").ap()
    d_wlT = nc.dram_tensor("wlT", [ATT, 1], F32R, kind="ExternalInput").ap()
    d_ones1 = nc.dram_tensor("ones1", [1, 128], F32R, kind="ExternalInput").ap()
    d_wkT = nc.dram_tensor("wkT", [CTX, ATT], BF16, kind="ExternalInput").ap()
    d_wqT = nc.dram_tensor("wqT", [HID, ATT], BF16, kind="ExternalInput").ap()
    d_w1T = nc.dram_tensor("w1T", [IN, H3], BF16, kind="ExternalInput").ap()
    d_w2T = nc.dram_tensor("w2T", [CTX, H3], BF16, kind="ExternalInput").ap()
    d_whhT = nc.dram_tensor("whhT", [HID, H3], BF16, kind="ExternalInput").ap()
    d_hT = nc.dram_tensor("hT", [HID, BL], BF16, kind="ExternalInput").ap()
    d_inT = nc.dram_tensor("inT", [IN, BL], BF16, kind="ExternalInput").ap()
    d_identb = nc.dram_tensor("identb", [128, 128], BF16, kind="ExternalInput").ap()
    d_identf = nc.dram_tensor("identf", [128, 128], F32, kind="ExternalInput").ap()
    d_one1 = nc.dram_tensor("one1", [1, 1], F32, kind="ExternalInput").ap()
    d_hTf = nc.dram_tensor("hTf", [HID, BL], F32, kind="ExternalInput").ap()
    d_bqk = nc.dram_tensor("bqk", [ATT, 1], F32, kind="ExternalInput").ap()
    d_bx = nc.dram_tensor("bx", [H3, 1], F32, kind="ExternalInput").ap()
    d_bhh = nc.dram_tensor("bhh", [H3, 1], F32, kind="ExternalInput").ap()

    d_h1 = nc.dram_tensor("h1", [BL, HID], F32, kind="ExternalOutput").ap()
    d_attn = nc.dram_tensor("attn", [BL, CTX], F32, kind="ExternalOutput").ap()

    with tile.TileContext(nc) as tc:
        _emit(tc, locals())
    nc.compile()
    return nc


def _emit(tc, d):
    from contextlib import ExitStack

    nc = tc.nc
    AT, CT, HT, H3T = ATT // 128, CTX // 128, HID // 128, H3 // 128  # 8,8,8,24
    ST = S // 128

    stack = ExitStack()
    pool = lambda *a, **k: stack.enter_context(tc.tile_pool(*a, **k))
    cst = pool(name="cst", bufs=1)
    actp = pool(name="actp", bufs=1)
    wkp = pool(name="wkp", bufs=1)
    wstream = pool(name="wstream", bufs=10)
    natp = pool(name="natp", bufs=16)
    natbp = pool(name="natbp", bufs=10)
    ctxTp = pool(name="ctxTp", bufs=12)
    tanhp = pool(name="tanhp", bufs=4)
    expp = pool(name="expp", bufs=2)
    rowp = pool(name="rowp", bufs=3)
    arowp = pool(name="arowp", bufs=2)
    ecolp = pool(name="ecolp", bufs=2)
    smallp = pool(name="smallp", bufs=6)
    gatep = pool(name="gatep", bufs=6)

    # PSUM pools: 8 banks total (pc 4 + pl 2 + shared scratch 2)
    pc_ps = pool(name="pc_ps", bufs=4, space="PSUM")
    pl_ps = pool(name="pl_ps", bufs=2, space="PSUM")
    ms_ps = pool(name="ms_ps", bufs=2, space="PSUM")

    # ---- constants ----
    identb = cst.tile([128, 128], BF16, tag="identb")
    nc.sync.dma_start(identb[:], d["d_identb"][:])
    identf = cst.tile([128, 128], F32, tag="identf")
    nc.sync.dma_start(identf[:], d["d_identf"][:])
    ones1 = cst.tile([1, 128], F32R, tag="ones1")
    nc.sync.dma_start(ones1[:], d["d_ones1"][:])
    one1 = cst.tile([1, 1], F32, tag="one1")
    nc.sync.dma_start(one1[:], d["d_one1"][:])

    wl_sb = cst.tile([128, AT], F32R, tag="wl")
    for a in range(AT):
        nc.sync.dma_start(wl_sb[:, a : a + 1], d["d_wlT"][128 * a : 128 * (a + 1), :])
    bqk_sb = cst.tile([128, AT], F32, tag="bqk")
    for a in range(AT):
        nc.sync.dma_start(bqk_sb[:, a : a + 1], d["d_bqk"][128 * a : 128 * (a + 1), :])
    bx_sb = cst.tile([128, H3T], F32, tag="bx")
    bhh_sb = cst.tile([128, H3T], F32, tag="bhh")
    for t in range(H3T):
        nc.sync.dma_start(bx_sb[:, t : t + 1], d["d_bx"][128 * t : 128 * (t + 1), :])
        nc.sync.dma_start(bhh_sb[:, t : t + 1], d["d_bhh"][128 * t : 128 * (t + 1), :])

    hT_sb = actp.tile([128, HT * BL], BF16, tag="hT")
    inT_sb = actp.tile([128, HT * BL], BF16, tag="inT")
    hTf_sb = actp.tile([128, HT * BL], F32, tag="hTf")
    for t in range(HT):
        nc.sync.dma_start(
            hT_sb[:, t * BL : (t + 1) * BL], d["d_hT"][128 * t : 128 * (t + 1), :]
        )
        nc.sync.dma_start(
            inT_sb[:, t * BL : (t + 1) * BL], d["d_inT"][128 * t : 128 * (t + 1), :]
        )
        nc.sync.dma_start(
            hTf_sb[:, t * BL : (t + 1) * BL], d["d_hTf"][128 * t : 128 * (t + 1), :]
        )

    # Wk.T resident in bf16: block c -> wk_sb[:, c*ATT : (c+1)*ATT]
    wk_sb = wkp.tile([128, CT * ATT], BF16, tag="wk")
    for c in range(CT):
        nc.sync.dma_start(
            wk_sb[:, c * ATT : (c + 1) * ATT], d["d_wkT"][128 * c : 128 * (c + 1), :]
        )

    # ---- prologue: qeff = Wq@hiddenT + (bq+bk); gx1 = W1@inT + bx; gh = Whh@hT + bhh
    qeff = actp.tile([128, AT * BL], F32, tag="qeff")
    gx1 = actp.tile([128, H3T * BL], F32, tag="gx1")
    gh = actp.tile([128, H3T * BL], F32, tag="gh")
    for dst, nt, wname, bias_sb, rhs in (
        (qeff, AT, "d_wqT", bqk_sb, hT_sb),
        (gx1, H3T, "d_w1T", bx_sb, inT_sb),
        (gh, H3T, "d_whhT", bhh_sb, hT_sb),
    ):
        for t in range(nt):
            pg = ms_ps.tile([128, BL], F32, tag="ms")
            for j in range(HT):
                wt = wstream.tile([128, 128], BF16, tag="ws")
                nc.sync.dma_start(
                    wt[:], d[wname][128 * j : 128 * (j + 1), 128 * t : 128 * (t + 1)]
                )
                nc.tensor.matmul(
                    pg[:], wt[:], rhs[:, j * BL : (j + 1) * BL],
                    start=(j == 0), stop=(j == HT - 1),
                )
            nc.scalar.activation(
                dst[:, t * BL : (t + 1) * BL], pg[:], AF.Identity,
                bias=bias_sb[:, t : t + 1],
            )

    # ---- main attention loop over local batch rows ----
    # Each batch row's softmax/attention tail is deferred and emitted at
    # checkpoints inside the NEXT row's cache-matmul loop so the PE's
    # in-order stream always has dense matmul work while DVE/ACT chase
    # the softmax dependency chain.
    sums = actp.tile([128, BL], F32, tag="sums")
    recip = actp.tile([128, BL], F32, tag="recip")
    attnT = actp.tile([128, CT * BL], BF16, tag="attnT")
    deferred = []  # closures carrying batch b-1's softmax/attn chunks

    def make_chunks(b, nat, lrow, prow):
        state = {}

        def run1():  # broadcast + max + exp
            pb0 = ms_ps.tile([128, 512], F32, tag="ms")
            pb1 = ms_ps.tile([128, 512], F32, tag="ms")
            for pb, sl in ((pb0, slice(0, 512)), (pb1, slice(512, 1024))):
                nc.tensor.matmul(pb[:], ones1[:], lrow[0:1, sl], start=True, stop=False)
                nc.tensor.matmul(pb[:], ones1[:], prow[0:1, sl], start=False, stop=True)
            mx2 = smallp.tile([128, 2], F32, tag="mx2")
            nc.vector.reduce_max(mx2[:, 0:1], pb0[:], axis=AX.X)
            nc.vector.reduce_max(mx2[:, 1:2], pb1[:], axis=AX.X)
            nmx = smallp.tile([128, 1], F32, tag="nmx")
            nc.vector.reduce_max(nmx[:], mx2[:], axis=AX.X, negate=True)
            acc2 = smallp.tile([128, 2], F32, tag="acc2")
            etile = expp.tile([128, S], F32, tag="exp")
            nc.scalar.activation(
                etile[:, 0:512], pb0[:], AF.Exp, bias=nmx[:], accum_out=acc2[:, 0:1]
            )
            nc.scalar.activation(
                etile[:, 512:1024], pb1[:], AF.Exp, bias=nmx[:], accum_out=acc2[:, 1:2]
            )
            nc.vector.tensor_add(sums[:, b : b + 1], acc2[:, 0:1], acc2[:, 1:2])
            nc.vector.reciprocal(recip[:, b : b + 1], sums[:, b : b + 1])
            state["etile"] = etile

        def run2():  # exp row -> column layout
            etile = state["etile"]
            pe = ms_ps.tile([128, ST], F32, tag="ms")
            for st in range(ST):
                nc.tensor.matmul(
                    pe[:, st : st + 1],
                    etile[0:1, 128 * st : 128 * (st + 1)],
                    one1[:],
                    start=True, stop=True,
                )
            ecol = ecolp.tile([128, ST], F32R, tag="ecol")
            nc.vector.tensor_copy(ecol[:], pe[:])
            state["ecol"] = ecol

        def run3():  # attention values + normalized output row
            ecol = state["ecol"]
            arow = arowp.tile([1, CTX], F32, tag="arow")
            for cg in range(2):
                pav = ms_ps.tile([1, 512], F32, tag="ms")
                for st in range(ST):
                    nc.tensor.matmul(
                        pav[:], ecol[:, st : st + 1],
                        _r(nat[st][:, 512 * cg : 512 * (cg + 1)]),
                        start=(st == 0), stop=(st == ST - 1),
                    )
                nc.vector.tensor_copy(arow[:, 512 * cg : 512 * (cg + 1)], pav[:])
            an = rowp.tile([1, CTX], F32, tag="row")
            nc.vector.tensor_scalar_mul(an[:], arow[:], recip[0:1, b : b + 1])
            nc.sync.dma_start(d["d_attn"][b : b + 1, :], an[:])
            state["arow"] = arow

        def run4():  # attnT columns for the W2 matmul
            arow = state["arow"]
            pat = ms_ps.tile([128, CT], F32, tag="ms")
            for c in range(CT):
                nc.tensor.matmul(
                    pat[:, c : c + 1],
                    arow[0:1, 128 * c : 128 * (c + 1)],
                    one1[:],
                    start=True, stop=True,
                )
            for c in range(CT):
                nc.vector.tensor_copy(
                    attnT[:, c * BL + b : c * BL + b + 1], pat[:, c : c + 1]
                )

        return [run1, run2, run3, run4]

    for b in range(BL):
        # natural context tiles [s_part, c_free] in f32r + bf16 copies
        nat, natb = [], []
        for st in range(ST):
            t = natp.tile([128, CTX], F32R, tag="nat")
            nc.sync.dma_start(t[:], d["d_ctx"][b, 128 * st : 128 * (st + 1), :])
            nat.append(t)
            tb = natbp.tile([128, CTX], BF16, tag="natb")
            nc.gpsimd.tensor_copy(tb[:], t[:].bitcast(F32))
            natb.append(tb)

        # transpose to [c_part, s_free] via bf16 identity-matmuls
        ctxT = []
        for c in range(CT):
            tT = ctxTp.tile([128, S], BF16, tag="ctxT")
            for sg in range(2):
                pt = ms_ps.tile([128, 512], F32, tag="ms")
                for ss in range(4):
                    st = 4 * sg + ss
                    nc.tensor.matmul(
                        pt[:, 128 * ss : 128 * (ss + 1)],
                        natb[st][:, 128 * c : 128 * (c + 1)],
                        identb[:],
                        start=True, stop=True,
                    )
                nc.vector.tensor_copy(tT[:, 512 * sg : 512 * (sg + 1)], pt[:])
            ctxT.append(tT)

        # cache matmul + tanh + Wl reduction; pl matmuls for a-1 emitted
        # after the cache matmuls of a so the PE never waits on tanh.
        pl0 = pl_ps.tile([1, 512], F32, tag="pl")
        pl1 = pl_ps.tile([1, 512], F32, tag="pl")
        pending = None

        def emit_pl(th0, th1, a, pl0=pl0, pl1=pl1):
            nc.tensor.matmul(
                pl0[:], wl_sb[:, a : a + 1], _r(th0[:]),
                start=(a == 0), stop=(a == AT - 1),
            )
            nc.tensor.matmul(
                pl1[:], wl_sb[:, a : a + 1], _r(th1[:]),
                start=(a == 0), stop=(a == AT - 1),
            )

        for a in range(AT):
            pc0 = pc_ps.tile([128, 512], F32, tag="pc")
            pc1 = pc_ps.tile([128, 512], F32, tag="pc")
            for c in range(CT):
                lhs = wk_sb[:, c * ATT + 128 * a : c * ATT + 128 * (a + 1)]
                nc.tensor.matmul(
                    pc0[:], lhs, ctxT[c][:, 0:512],
                    start=(c == 0), stop=(c == CT - 1),
                )
                nc.tensor.matmul(
                    pc1[:], lhs, ctxT[c][:, 512:1024],
                    start=(c == 0), stop=(c == CT - 1),
                )
            if deferred and a in (0, 2, 4, 6):
                deferred[a // 2]()
            if pending is not None:
                emit_pl(*pending)
            th0 = tanhp.tile([128, 512], F32R, tag="tanh")
            th1 = tanhp.tile([128, 512], F32R, tag="tanh")
            qcol = qeff[:, a * BL + b : a * BL + b + 1]
            nc.scalar.activation(th0[:], pc0[:], AF.Tanh, bias=qcol)
            nc.scalar.activation(th1[:], pc1[:], AF.Tanh, bias=qcol)
            pending = (th0, th1, a)
        emit_pl(*pending)

        lrow = rowp.tile([1, S], F32R, tag="lrow")
        nc.vector.tensor_copy(lrow[:, 0:512], _r(pl0[:]))
        nc.vector.tensor_copy(lrow[:, 512:1024], _r(pl1[:]))
        prow = rowp.tile([1, S], F32R, tag="lrow")
        nc.sync.dma_start(prow[:], d["d_pen"][b : b + 1, :])

        deferred = make_chunks(b, nat, lrow, prow)

    for fn in deferred:  # flush last batch row
        fn()

    # ---- tail: gxa = W2 @ attnT (columns scaled by 1/sum); gates; hidden1
    r_all = actp.tile([128, HT * BL], F32, tag="r_all")
    z_all = actp.tile([128, HT * BL], F32, tag="z_all")
    h1nat = actp.tile([BL, HID], F32, tag="h1nat")
    for t in range(H3T):
        pg = ms_ps.tile([128, BL], F32, tag="ms")
        for c in range(CT):
            wt = wstream.tile([128, 128], BF16, tag="ws")
            nc.sync.dma_start(
                wt[:], d["d_w2T"][128 * c : 128 * (c + 1), 128 * t : 128 * (t + 1)]
            )
            nc.tensor.matmul(
                pg[:], wt[:], attnT[:, c * BL : (c + 1) * BL],
                start=(c == 0), stop=(c == CT - 1),
            )
        gxa = gatep.tile([128, BL], F32, tag="gxa")
        nc.vector.tensor_mul(gxa[:], pg[:], recip[:])
        gx = gatep.tile([128, BL], F32, tag="gx")
        nc.vector.tensor_add(gx[:], gxa[:], gx1[:, t * BL : (t + 1) * BL])
        ht = t % HT
        sl = slice(ht * BL, (ht + 1) * BL)
        if t < HT:  # r gate
            gs = gatep.tile([128, BL], F32, tag="gs")
            nc.vector.tensor_add(gs[:], gx[:], gh[:, t * BL : (t + 1) * BL])
            nc.scalar.activation(r_all[:, sl], gs[:], AF.Sigmoid)
        elif t < 2 * HT:  # z gate
            gs = gatep.tile([128, BL], F32, tag="gs")
            nc.vector.tensor_add(gs[:], gx[:], gh[:, t * BL : (t + 1) * BL])
            nc.scalar.activation(z_all[:, sl], gs[:], AF.Sigmoid)
        else:  # n gate, then hidden1 for this h-tile
            rh = gatep.tile([128, BL], F32, tag="rh")
            nc.vector.tensor_mul(rh[:], r_all[:, sl], gh[:, t * BL : (t + 1) * BL])
            ns = gatep.tile([128, BL], F32, tag="ns")
            nc.vector.tensor_add(ns[:], gx[:], rh[:])
            ntl = gatep.tile([128, BL], F32, tag="ntl")
            nc.scalar.activation(ntl[:], ns[:], AF.Tanh)
            hmn = gatep.tile([128, BL], F32, tag="hmn")
            nc.vector.tensor_sub(hmn[:], hTf_sb[:, sl], ntl[:])
            zh = gatep.tile([128, BL], F32, tag="zh")
            nc.vector.tensor_mul(zh[:], z_all[:, sl], hmn[:])
            h1T = gatep.tile([128, BL], F32, tag="h1T")
            nc.vector.tensor_add(h1T[:], ntl[:], zh[:])
            ph = ms_ps.tile([BL, 128], F32, tag="ms")
            nc.tensor.transpose(ph[:], h1T[:], identf[:])
            nc.vector.tensor_copy(h1nat[:, 128 * ht : 128 * (ht + 1)], ph[:])
    nc.sync.dma_start(d["d_h1"][:], h1nat[:])
    stack.close()


_NC_CACHE = None


def _get_program():
    global _NC_CACHE
    if _NC_CACHE is None:
        _NC_CACHE = build_program()
    return _NC_CACHE


def make_in_maps(inputs):
    """Host-side prep: shard batch across cores, transpose/fuse weights."""
    f = lambda x: np.ascontiguousarray(np.asarray(x, dtype=np.float32))
    bf = lambda x: np.ascontiguousarray(np.asarray(x, dtype=np.float32).astype(BF16NP))
    input_ = f(inputs["input"])
    hidden = f(inputs["hidden"])
    context = f(inputs["context"])
    mask = np.asarray(inputs["context_mask"])
    Wq, bq = f(inputs["Wq"]), f(inputs["bq"])
    Wk, bk = f(inputs["Wk"]), f(inputs["bk"])
    Wl = f(inputs["Wl"])
    We, be = f(inputs["We"]), f(inputs["be"])
    Wa, ba = f(inputs["Wa"]), f(inputs["ba"])
    W_ih, W_hh = f(inputs["W_ih"]), f(inputs["W_hh"])
    b_ih, b_hh = f(inputs["b_ih"]), f(inputs["b_hh"])

    shared = {
        "wkT": bf(Wk.T),
        "wqT": bf(Wq.T),
        "w1T": bf((W_ih @ We).T),
        "w2T": bf((W_ih @ Wa).T),
        "whhT": bf(W_hh.T),
        "wlT": np.ascontiguousarray(Wl.T),
        "bqk": np.ascontiguousarray((bq + bk).reshape(ATT, 1)),
        "bx": np.ascontiguousarray((W_ih @ (be + ba) + b_ih).reshape(H3, 1)),
        "bhh": np.ascontiguousarray(b_hh.reshape(H3, 1)),
        "identb": np.eye(128, dtype=BF16NP),
        "identf": np.eye(128, dtype=np.float32),
        "ones1": np.ones((1, 128), np.float32),
        "one1": np.ones((1, 1), np.float32),
    }
    pen = np.where(mask, np.float32(-1e18), np.float32(0.0)).astype(np.float32)
    inT = np.ascontiguousarray(input_.T)
    hT = np.ascontiguousarray(hidden.T)

    in_maps = []
    for k in range(NCORES):
        sl = slice(k * BL, (k + 1) * BL)
        in_maps.append(
            {
                "ctx": context[sl],
                "pen": np.ascontiguousarray(pen[sl]),
                "inT": np.ascontiguousarray(inT[:, sl]).astype(BF16NP),
                "hT": np.ascontiguousarray(hT[:, sl]).astype(BF16NP),
                "hTf": np.ascontiguousarray(hT[:, sl]),
                **shared,
            }
        )
    return in_maps


def kernel(**inputs):
    nc = _get_program()
    in_maps = make_in_maps(inputs)
    res = run_bass_kernel_spmd(nc, in_maps, core_ids=list(range(NCORES)))
    hidden1 = np.concatenate([res.results[k]["h1"] for k in range(NCORES)], axis=0)
    attn = np.concatenate([res.results[k]["attn"] for k in range(NCORES)], axis=0)
    return (hidden1, attn)


# revision 20
# speedup vs baseline: 1.7461x; 1.7461x over previous
"""Trainium2 Bass kernel for an attention-augmented GRU cell (CGRUCell).

Reference computation (per batch row):
    cache   = context @ Wk.T + bk                  # [S, A]
    q       = hidden @ Wq.T + bq                   # [A]
    logits  = tanh(q + cache) @ Wl[0] + bl         # [S]
    logits  = where(mask, -1e18, logits)
    w       = softmax(logits)                      # [S]
    attn    = w @ context                          # [CTX]
    x       = input @ We.T + be + attn @ Wa.T + ba
    gx      = x @ W_ih.T + b_ih ; gh = hidden @ W_hh.T + b_hh
    r, z    = sigmoid(gx_r + gh_r), sigmoid(gx_z + gh_z)
    n       = tanh(gx_n + r * gh_n)
    hidden1 = (1 - z) * n + z * hidden
Outputs: (hidden1, attn)

Strategy: data-parallel over batch on 8 NeuronCores (8 rows each). The
dominant work is the [S,CTX]@[CTX,A] key projection; it runs on the
TensorEngine in bf16 (context is cast on the otherwise-idle GpSimd
engine, then transposed with cheap bf16 identity-matmuls so the
contraction dim lands on partitions). The softmax reduction over the
attention dim rides on f32r matmuls against Wl with the q/bk bias fused
into the tanh Activation op; the attention-value matvec contracts the
full-precision f32r context so the attn output keeps ~1e-4 accuracy.
The GRU algebra is reassociated (W1 = W_ih@We, W2 = W_ih@Wa) so all of
it except attn @ W2.T is computed in a prologue from the raw inputs.
The softmax/attention tail of each batch row is emitted interleaved
into the next row's cache matmuls so the in-order PE never idles on
DVE/ACT latency.
"""

import sys

if "/opt/trn_rl_repo" not in sys.path:
    sys.path.insert(0, "/opt/trn_rl_repo")

import ml_dtypes
import numpy as np

import concourse.bass as bass
import concourse.tile as tile
from concourse import bacc, mybir
from concourse.bass_utils import run_bass_kernel_spmd

NCORES = 8
B, S, IN, HID, CTX, ATT = 64, 1024, 1024, 1024, 1024, 1024
BL = B // NCORES          # batch rows per core
H3 = 3 * HID
F32 = mybir.dt.float32
F32R = mybir.dt.float32r
BF16 = mybir.dt.bfloat16
AX = mybir.AxisListType
AF = mybir.ActivationFunctionType
BF16NP = ml_dtypes.bfloat16


def _r(ap):
    return ap.bitcast(F32R)


def build_program():
    nc = bacc.Bacc("TRN2", target_bir_lowering=False, debug=False, num_devices=NCORES)

    d_ctx = nc.dram_tensor("ctx", [BL, S, CTX], F32R, kind="ExternalInput").ap()
    d_pen = nc.dram_tensor("pen", [BL, S], F32R, kind="ExternalInput").ap()
    d_wlT = nc.dram_tensor("wlT", [ATT, 1], F32R, kind="ExternalInput").ap()
    d_ones1 = nc.dram_tensor("ones1", [1, 128], F32R, kind="ExternalInput").ap()
    d_wkT = nc.dram_tensor("wkT", [CTX, ATT], BF16, kind="ExternalInput").ap()
    d_wqT = nc.dram_tensor("wqT", [HID, ATT], BF16, kind="ExternalInput").ap()
    d_w1T = nc.dram_tensor("w1T", [IN, H3], BF16, kind="ExternalInput").ap()
    d_w2T = nc.dram_tensor("w2T", [CTX, H3], BF16, kind="ExternalInput").ap()
    d_whhT = nc.dram_tensor("whhT", [HID, H3], BF16, kind="ExternalInput").ap()
    d_hT = nc.dram_tensor("hT", [HID, BL], BF16, kind="ExternalInput").ap()
    d_inT = nc.dram_tensor("inT", [IN, BL], BF16, kind="ExternalInput").ap()
    d_identb = nc.dram_tensor("identb", [128, 128], BF16, kind="ExternalInput").ap()
    d_identf = nc.dram_tensor("identf", [128, 128], F32, kind="ExternalInput").ap()
    d_one1 = nc.dram_tensor("one1", [1, 1], F32, kind="ExternalInput").ap()
    d_hTf = nc.dram_tensor("hTf", [HID, BL], F32, kind="ExternalInput").ap()
    d_bqk = nc.dram_tensor("bqk", [ATT, 1], F32, kind="ExternalInput").ap()
    d_bx = nc.dram_tensor("bx", [H3, 1], F32, kind="ExternalInput").ap()
    d_bhh = nc.dram_tensor("bhh", [H3, 1], F32, kind="ExternalInput").ap()

    d_h1 = nc.dram_tensor("h1", [BL, HID], F32, kind="ExternalOutput").ap()
    d_attn = nc.dram_tensor("attn", [BL, CTX], F32, kind="ExternalOutput").ap()

    with tile.TileContext(nc) as tc:
        _emit(tc, locals())
    nc.compile()
    return nc


def _emit(tc, d):
    from contextlib import ExitStack

    nc = tc.nc
    AT, CT, HT, H3T = ATT // 128, CTX // 128, HID // 128, H3 // 128  # 8,8,8,24
    ST = S // 128

    stack = ExitStack()
    pool = lambda *a, **k: stack.enter_context(tc.tile_pool(*a, **k))
    cst = pool(name="cst", bufs=1)
    actp = pool(name="actp", bufs=1)
    wkp = pool(name="wkp", bufs=1)
    wstream = pool(name="wstream", bufs=10)
    natp = pool(name="natp", bufs=16)
    natbp = pool(name="natbp", bufs=10)
    ctxTp = pool(name="ctxTp", bufs=12)
    tanhp = pool(name="tanhp", bufs=4)
    expp = pool(name="expp", bufs=2)
    rowp = pool(name="rowp", bufs=3)
    arowp = pool(name="arowp", bufs=2)
    ecolp = pool(name="ecolp", bufs=2)
    smallp = pool(name="smallp", bufs=6)
    gatep = pool(name="gatep", bufs=6)

    # PSUM pools: 8 banks total (pc 4 + pl 2 + shared scratch 2)
    pc_ps = pool(name="pc_ps", bufs=4, space="PSUM")
    pl_ps = pool(name="pl_ps", bufs=2, space="PSUM")
    ms_ps = pool(name="ms_ps", bufs=2, space="PSUM")

    # ---- constants ----
    identb = cst.tile([128, 128], BF16, tag="identb")
    nc.sync.dma_start(identb[:], d["d_identb"][:])
    identf = cst.tile([128, 128], F32, tag="identf")
    nc.sync.dma_start(identf[:], d["d_identf"][:])
    ones1 = cst.tile([1, 128], F32R, tag="ones1")
    nc.sync.dma_start(ones1[:], d["d_ones1"][:])
    one1 = cst.tile([1, 1], F32, tag="one1")
    nc.sync.dma_start(one1[:], d["d_one1"][:])

    wl_sb = cst.tile([128, AT], F32R, tag="wl")
    nc.sync.dma_start(wl_sb[:], d["d_wlT"].rearrange("(t p) o -> p t o", p=128))
    bqk_sb = cst.tile([128, AT], F32, tag="bqk")
    nc.sync.dma_start(bqk_sb[:], d["d_bqk"].rearrange("(t p) o -> p t o", p=128))
    bx_sb = cst.tile([128, H3T], F32, tag="bx")
    bhh_sb = cst.tile([128, H3T], F32, tag="bhh")
    nc.sync.dma_start(bx_sb[:], d["d_bx"].rearrange("(t p) o -> p t o", p=128))
    nc.sync.dma_start(bhh_sb[:], d["d_bhh"].rearrange("(t p) o -> p t o", p=128))

    hT_sb = actp.tile([128, HT * BL], BF16, tag="hT")
    inT_sb = actp.tile([128, HT * BL], BF16, tag="inT")
    hTf_sb = actp.tile([128, HT * BL], F32, tag="hTf")
    nc.sync.dma_start(hT_sb[:], d["d_hT"].rearrange("(t p) b -> p t b", p=128))
    nc.sync.dma_start(inT_sb[:], d["d_inT"].rearrange("(t p) b -> p t b", p=128))
    nc.sync.dma_start(hTf_sb[:], d["d_hTf"].rearrange("(t p) b -> p t b", p=128))

    # Wk.T resident in bf16: block c -> wk_sb[:, c*ATT : (c+1)*ATT]
    wk_sb = wkp.tile([128, CT * ATT], BF16, tag="wk")
    for c in range(CT):
        nc.sync.dma_start(
            wk_sb[:, c * ATT : (c + 1) * ATT], d["d_wkT"][128 * c : 128 * (c + 1), :]
        )

    # ---- prologue: qeff = Wq@hiddenT + (bq+bk); gx1 = W1@inT + bx; gh = Whh@hT + bhh
    qeff = actp.tile([128, AT * BL], F32, tag="qeff")
    gx1 = actp.tile([128, H3T * BL], F32, tag="gx1")
    gh = actp.tile([128, H3T * BL], F32, tag="gh")
    TG = 4  # output tiles per weight DMA
    for dst, nt, wname, bias_sb, rhs in (
        (qeff, AT, "d_wqT", bqk_sb, hT_sb),
        (gx1, H3T, "d_w1T", bx_sb, inT_sb),
        (gh, H3T, "d_whhT", bhh_sb, hT_sb),
    ):
        for t0 in range(0, nt, TG):
            wt = wstream.tile([128, HT * TG * 128], BF16, tag="ws")
            nc.sync.dma_start(
                wt[:],
                d[wname][:, 128 * t0 : 128 * (t0 + TG)].rearrange(
                    "(j p) m -> p j m", p=128
                ),
            )
            for tl in range(TG):
                t = t0 + tl
                pg = ms_ps.tile([128, BL], F32, tag="ms")
                for j in range(HT):
                    lhs = wt[:, j * TG * 128 + tl * 128 : j * TG * 128 + (tl + 1) * 128]
                    nc.tensor.matmul(
                        pg[:], lhs, rhs[:, j * BL : (j + 1) * BL],
                        start=(j == 0), stop=(j == HT - 1),
                    )
                nc.scalar.activation(
                    dst[:, t * BL : (t + 1) * BL], pg[:], AF.Identity,
                    bias=bias_sb[:, t : t + 1],
                )

    # ---- main attention loop over local batch rows ----
    # Each batch row's softmax/attention tail is deferred and emitted at
    # checkpoints inside the NEXT row's cache-matmul loop so the PE's
    # in-order stream always has dense matmul work while DVE/ACT chase
    # the softmax dependency chain.
    sums = actp.tile([128, BL], F32, tag="sums")
    recip = actp.tile([128, BL], F32, tag="recip")
    attnT = actp.tile([128, CT * BL], BF16, tag="attnT")
    deferred = []  # closures carrying batch b-1's softmax/attn chunks

    def make_chunks(b, nat, lrow, prow):
        state = {}

        def run1():  # broadcast + max + exp
            pb0 = ms_ps.tile([128, 512], F32, tag="ms")
            pb1 = ms_ps.tile([128, 512], F32, tag="ms")
            for pb, sl in ((pb0, slice(0, 512)), (pb1, slice(512, 1024))):
                nc.tensor.matmul(pb[:], ones1[:], lrow[0:1, sl], start=True, stop=False)
                nc.tensor.matmul(pb[:], ones1[:], prow[0:1, sl], start=False, stop=True)
            mx2 = smallp.tile([128, 2], F32, tag="mx2")
            nc.vector.reduce_max(mx2[:, 0:1], pb0[:], axis=AX.X)
            nc.vector.reduce_max(mx2[:, 1:2], pb1[:], axis=AX.X)
            nmx = smallp.tile([128, 1], F32, tag="nmx")
            nc.vector.reduce_max(nmx[:], mx2[:], axis=AX.X, negate=True)
            acc2 = smallp.tile([128, 2], F32, tag="acc2")
            etile = expp.tile([128, S], F32, tag="exp")
            nc.scalar.activation(
                etile[:, 0:512], pb0[:], AF.Exp, bias=nmx[:], accum_out=acc2[:, 0:1]
            )
            nc.scalar.activation(
                etile[:, 512:1024], pb1[:], AF.Exp, bias=nmx[:], accum_out=acc2[:, 1:2]
            )
            nc.vector.tensor_add(sums[:, b : b + 1], acc2[:, 0:1], acc2[:, 1:2])
            nc.vector.reciprocal(recip[:, b : b + 1], sums[:, b : b + 1])
            state["etile"] = etile

        def run2():  # exp row -> column layout
            etile = state["etile"]
            pe = ms_ps.tile([128, ST], F32, tag="ms")
            for st in range(ST):
                nc.tensor.matmul(
                    pe[:, st : st + 1],
                    etile[0:1, 128 * st : 128 * (st + 1)],
                    one1[:],
                    start=True, stop=True,
                )
            ecol = ecolp.tile([128, ST], F32R, tag="ecol")
            nc.vector.tensor_copy(ecol[:], pe[:])
            state["ecol"] = ecol

        def run3():  # attention values + normalized output row
            ecol = state["ecol"]
            arow = arowp.tile([1, CTX], F32, tag="arow")
            for cg in range(2):
                pav = ms_ps.tile([1, 512], F32, tag="ms")
                for st in range(ST):
                    nc.tensor.matmul(
                        pav[:], ecol[:, st : st + 1],
                        _r(nat[st][:, 512 * cg : 512 * (cg + 1)]),
                        start=(st == 0), stop=(st == ST - 1),
                    )
                nc.vector.tensor_copy(arow[:, 512 * cg : 512 * (cg + 1)], pav[:])
            an = rowp.tile([1, CTX], F32, tag="row")
            nc.vector.tensor_scalar_mul(an[:], arow[:], recip[0:1, b : b + 1])
            nc.sync.dma_start(d["d_attn"][b : b + 1, :], an[:])
            state["arow"] = arow

        def run4():  # attnT columns for the W2 matmul
            arow = state["arow"]
            pat = ms_ps.tile([128, CT], F32, tag="ms")
            for c in range(CT):
                nc.tensor.matmul(
                    pat[:, c : c + 1],
                    arow[0:1, 128 * c : 128 * (c + 1)],
                    one1[:],
                    start=True, stop=True,
                )
            for c in range(CT):
                nc.vector.tensor_copy(
                    attnT[:, c * BL + b : c * BL + b + 1], pat[:, c : c + 1]
                )

        return [run1, run2, run3, run4]

    for b in range(BL):
        # natural context tiles [s_part, c_free] in f32r + bf16 copies
        nat, natb = [], []
        for st in range(ST):
            t = natp.tile([128, CTX], F32R, tag="nat")
            nc.scalar.dma_start(t[:], d["d_ctx"][b, 128 * st : 128 * (st + 1), :])
            nat.append(t)
            tb = natbp.tile([128, CTX], BF16, tag="natb")
            nc.gpsimd.dma_start(tb[:], t[:].bitcast(F32))
            natb.append(tb)

        # transpose to [c_part, s_free] via bf16 identity-matmuls
        ctxT = []
        for c in range(CT):
            tT = ctxTp.tile([128, S], BF16, tag="ctxT")
            for sg in range(2):
                pt = ms_ps.tile([128, 512], F32, tag="ms")
                for ss in range(4):
                    st = 4 * sg + ss
                    nc.tensor.matmul(
                        pt[:, 128 * ss : 128 * (ss + 1)],
                        natb[st][:, 128 * c : 128 * (c + 1)],
                        identb[:],
                        start=True, stop=True,
                    )
                nc.vector.tensor_copy(tT[:, 512 * sg : 512 * (sg + 1)], pt[:])
            ctxT.append(tT)

        # cache matmul + tanh + Wl reduction; pl matmuls for a-1 emitted
        # after the cache matmuls of a so the PE never waits on tanh.
        pl0 = pl_ps.tile([1, 512], F32, tag="pl")
        pl1 = pl_ps.tile([1, 512], F32, tag="pl")
        pending = None

        def emit_pl(th0, th1, a, pl0=pl0, pl1=pl1):
            nc.tensor.matmul(
                pl0[:], wl_sb[:, a : a + 1], _r(th0[:]),
                start=(a == 0), stop=(a == AT - 1),
            )
            nc.tensor.matmul(
                pl1[:], wl_sb[:, a : a + 1], _r(th1[:]),
                start=(a == 0), stop=(a == AT - 1),
            )

        for a in range(AT):
            pc0 = pc_ps.tile([128, 512], F32, tag="pc")
            pc1 = pc_ps.tile([128, 512], F32, tag="pc")
            for c in range(CT):
                lhs = wk_sb[:, c * ATT + 128 * a : c * ATT + 128 * (a + 1)]
                nc.tensor.matmul(
                    pc0[:], lhs, ctxT[c][:, 0:512],
                    start=(c == 0), stop=(c == CT - 1),
                )
                nc.tensor.matmul(
                    pc1[:], lhs, ctxT[c][:, 512:1024],
                    start=(c == 0), stop=(c == CT - 1),
                )
            if deferred and a in (0, 2, 4, 6):
                deferred[a // 2]()
            if pending is not None:
                emit_pl(*pending)
            th0 = tanhp.tile([128, 512], F32R, tag="tanh")
            th1 = tanhp.tile([128, 512], F32R, tag="tanh")
            qcol = qeff[:, a * BL + b : a * BL + b + 1]
            nc.scalar.activation(th0[:], pc0[:], AF.Tanh, bias=qcol)
            nc.scalar.activation(th1[:], pc1[:], AF.Tanh, bias=qcol)
            pending = (th0, th1, a)
        emit_pl(*pending)

        lrow = rowp.tile([1, S], F32R, tag="lrow")
        nc.vector.tensor_copy(lrow[:, 0:512], _r(pl0[:]))
        nc.vector.tensor_copy(lrow[:, 512:1024], _r(pl1[:]))
        prow = rowp.tile([1, S], F32R, tag="lrow")
        nc.sync.dma_start(prow[:], d["d_pen"][b : b + 1, :])

        deferred = make_chunks(b, nat, lrow, prow)

    for fn in deferred:  # flush last batch row
        fn()

    # ---- tail: gxa = W2 @ attnT (columns scaled by 1/sum); gates; hidden1
    r_all = actp.tile([128, HT * BL], F32, tag="r_all")
    z_all = actp.tile([128, HT * BL], F32, tag="z_all")
    h1nat = actp.tile([BL, HID], F32, tag="h1nat")
    for t0 in range(0, H3T, TG):
        wt2 = wstream.tile([128, CT * TG * 128], BF16, tag="ws")
        nc.sync.dma_start(
            wt2[:],
            d["d_w2T"][:, 128 * t0 : 128 * (t0 + TG)].rearrange(
                "(j p) m -> p j m", p=128
            ),
        )
        for tl in range(TG):
            t = t0 + tl
            pg = ms_ps.tile([128, BL], F32, tag="ms")
            for c in range(CT):
                lhs = wt2[:, c * TG * 128 + tl * 128 : c * TG * 128 + (tl + 1) * 128]
                nc.tensor.matmul(
                    pg[:], lhs, attnT[:, c * BL : (c + 1) * BL],
                    start=(c == 0), stop=(c == CT - 1),
                )
            gxa = gatep.tile([128, BL], F32, tag="gxa")
            nc.vector.tensor_mul(gxa[:], pg[:], recip[:])
            gx = gatep.tile([128, BL], F32, tag="gx")
            nc.vector.tensor_add(gx[:], gxa[:], gx1[:, t * BL : (t + 1) * BL])
            ht = t % HT
            sl = slice(ht * BL, (ht + 1) * BL)
            if t < HT:  # r gate
                gs = gatep.tile([128, BL], F32, tag="gs")
                nc.vector.tensor_add(gs[:], gx[:], gh[:, t * BL : (t + 1) * BL])
                nc.scalar.activation(r_all[:, sl], gs[:], AF.Sigmoid)
            elif t < 2 * HT:  # z gate
                gs = gatep.tile([128, BL], F32, tag="gs")
                nc.vector.tensor_add(gs[:], gx[:], gh[:, t * BL : (t + 1) * BL])
                nc.scalar.activation(z_all[:, sl], gs[:], AF.Sigmoid)
            else:  # n gate, then hidden1 for this h-tile
                rh = gatep.tile([128, BL], F32, tag="rh")
                nc.vector.tensor_mul(rh[:], r_all[:, sl], gh[:, t * BL : (t + 1) * BL])
                ns = gatep.tile([128, BL], F32, tag="ns")
                nc.vector.tensor_add(ns[:], gx[:], rh[:])
                ntl = gatep.tile([128, BL], F32, tag="ntl")
                nc.scalar.activation(ntl[:], ns[:], AF.Tanh)
                hmn = gatep.tile([128, BL], F32, tag="hmn")
                nc.vector.tensor_sub(hmn[:], hTf_sb[:, sl], ntl[:])
                zh = gatep.tile([128, BL], F32, tag="zh")
                nc.vector.tensor_mul(zh[:], z_all[:, sl], hmn[:])
                h1T = gatep.tile([128, BL], F32, tag="h1T")
                nc.vector.tensor_add(h1T[:], ntl[:], zh[:])
                ph = ms_ps.tile([BL, 128], F32, tag="ms")
                nc.tensor.transpose(ph[:], h1T[:], identf[:])
                nc.vector.tensor_copy(h1nat[:, 128 * ht : 128 * (ht + 1)], ph[:])
    nc.sync.dma_start(d["d_h1"][:], h1nat[:])
    stack.close()


_NC_CACHE = None


def _get_program():
    global _NC_CACHE
    if _NC_CACHE is None:
        _NC_CACHE = build_program()
    return _NC_CACHE


def make_in_maps(inputs):
    """Host-side prep: shard batch across cores, transpose/fuse weights."""
    f = lambda x: np.ascontiguousarray(np.asarray(x, dtype=np.float32))
    bf = lambda x: np.ascontiguousarray(np.asarray(x, dtype=np.float32).astype(BF16NP))
    input_ = f(inputs["input"])
    hidden = f(inputs["hidden"])
    context = f(inputs["context"])
    mask = np.asarray(inputs["context_mask"])
    Wq, bq = f(inputs["Wq"]), f(inputs["bq"])
    Wk, bk = f(inputs["Wk"]), f(inputs["bk"])
    Wl = f(inputs["Wl"])
    We, be = f(inputs["We"]), f(inputs["be"])
    Wa, ba = f(inputs["Wa"]), f(inputs["ba"])
    W_ih, W_hh = f(inputs["W_ih"]), f(inputs["W_hh"])
    b_ih, b_hh = f(inputs["b_ih"]), f(inputs["b_hh"])

    shared = {
        "wkT": bf(Wk.T),
        "wqT": bf(Wq.T),
        "w1T": bf((W_ih @ We).T),
        "w2T": bf((W_ih @ Wa).T),
        "whhT": bf(W_hh.T),
        "wlT": np.ascontiguousarray(Wl.T),
        "bqk": np.ascontiguousarray((bq + bk).reshape(ATT, 1)),
        "bx": np.ascontiguousarray((W_ih @ (be + ba) + b_ih).reshape(H3, 1)),
        "bhh": np.ascontiguousarray(b_hh.reshape(H3, 1)),
        "identb": np.eye(128, dtype=BF16NP),
        "identf": np.eye(128, dtype=np.float32),
        "ones1": np.ones((1, 128), np.float32),
        "one1": np.ones((1, 1), np.float32),
    }
    pen = np.where(mask, np.float32(-1e18), np.float32(0.0)).astype(np.float32)
    inT = np.ascontiguousarray(input_.T)
    hT = np.ascontiguousarray(hidden.T)

    in_maps = []
    for k in range(NCORES):
        sl = slice(k * BL, (k + 1) * BL)
        in_maps.append(
            {
                "ctx": context[sl],
                "pen": np.ascontiguousarray(pen[sl]),
                "inT": np.ascontiguousarray(inT[:, sl]).astype(BF16NP),
                "hT": np.ascontiguousarray(hT[:, sl]).astype(BF16NP),
                "hTf": np.ascontiguousarray(hT[:, sl]),
                **shared,
            }
        )
    return in_maps


def kernel(**inputs):
    nc = _get_program()
    in_maps = make_in_maps(inputs)
    res = run_bass_kernel_spmd(nc, in_maps, core_ids=list(range(NCORES)))
    hidden1 = np.concatenate([res.results[k]["h1"] for k in range(NCORES)], axis=0)
    attn = np.concatenate([res.results[k]["attn"] for k in range(NCORES)], axis=0)
    return (hidden1, attn)


# revision 21
# speedup vs baseline: 1.8203x; 1.0425x over previous
"""Trainium2 Bass kernel for an attention-augmented GRU cell (CGRUCell).

Reference computation (per batch row):
    cache   = context @ Wk.T + bk                  # [S, A]
    q       = hidden @ Wq.T + bq                   # [A]
    logits  = tanh(q + cache) @ Wl[0] + bl         # [S]
    logits  = where(mask, -1e18, logits)
    w       = softmax(logits)                      # [S]
    attn    = w @ context                          # [CTX]
    x       = input @ We.T + be + attn @ Wa.T + ba
    gx      = x @ W_ih.T + b_ih ; gh = hidden @ W_hh.T + b_hh
    r, z    = sigmoid(gx_r + gh_r), sigmoid(gx_z + gh_z)
    n       = tanh(gx_n + r * gh_n)
    hidden1 = (1 - z) * n + z * hidden
Outputs: (hidden1, attn)

Strategy: data-parallel over batch on 8 NeuronCores (8 rows each). The
dominant work is the [S,CTX]@[CTX,A] key projection; it runs on the
TensorEngine in bf16 (context is cast on the otherwise-idle GpSimd
engine, then transposed with cheap bf16 identity-matmuls so the
contraction dim lands on partitions). The softmax reduction over the
attention dim rides on f32r matmuls against Wl with the q/bk bias fused
into the tanh Activation op; the attention-value matvec contracts the
full-precision f32r context so the attn output keeps ~1e-4 accuracy.
The GRU algebra is reassociated (W1 = W_ih@We, W2 = W_ih@Wa) so all of
it except attn @ W2.T is computed in a prologue from the raw inputs.
The softmax/attention tail of each batch row is emitted interleaved
into the next row's cache matmuls so the in-order PE never idles on
DVE/ACT latency.
"""

import sys

if "/opt/trn_rl_repo" not in sys.path:
    sys.path.insert(0, "/opt/trn_rl_repo")

import ml_dtypes
import numpy as np

import concourse.bass as bass
import concourse.tile as tile
from concourse import bacc, mybir
from concourse.bass_utils import run_bass_kernel_spmd

NCORES = 8
B, S, IN, HID, CTX, ATT = 64, 1024, 1024, 1024, 1024, 1024
BL = B // NCORES          # batch rows per core
H3 = 3 * HID
F32 = mybir.dt.float32
F32R = mybir.dt.float32r
BF16 = mybir.dt.bfloat16
AX = mybir.AxisListType
AF = mybir.ActivationFunctionType
BF16NP = ml_dtypes.bfloat16


def _r(ap):
    return ap.bitcast(F32R)


def build_program():
    nc = bacc.Bacc("TRN2", target_bir_lowering=False, debug=False, num_devices=NCORES)

    d_ctx = nc.dram_tensor("ctx", [BL, S, CTX], F32R, kind="ExternalInput").ap()
    d_pen = nc.dram_tensor("pen", [BL, S], F32R, kind="ExternalInput").ap()
    d_wlT = nc.dram_tensor("wlT", [ATT, 1], F32R, kind="ExternalInput").ap()
    d_ones1 = nc.dram_tensor("ones1", [1, 128], F32R, kind="ExternalInput").ap()
    d_wkT = nc.dram_tensor("wkT", [CTX, ATT], BF16, kind="ExternalInput").ap()
    d_wqT = nc.dram_tensor("wqT", [HID, ATT], BF16, kind="ExternalInput").ap()
    d_w1T = nc.dram_tensor("w1T", [IN, H3], BF16, kind="ExternalInput").ap()
    d_w2T = nc.dram_tensor("w2T", [CTX, H3], BF16, kind="ExternalInput").ap()
    d_whhT = nc.dram_tensor("whhT", [HID, H3], BF16, kind="ExternalInput").ap()
    d_hT = nc.dram_tensor("hT", [HID, BL], BF16, kind="ExternalInput").ap()
    d_inT = nc.dram_tensor("inT", [IN, BL], BF16, kind="ExternalInput").ap()
    d_identb = nc.dram_tensor("identb", [128, 128], BF16, kind="ExternalInput").ap()
    d_identf = nc.dram_tensor("identf", [128, 128], F32, kind="ExternalInput").ap()
    d_one1 = nc.dram_tensor("one1", [1, 1], F32, kind="ExternalInput").ap()
    d_hTf = nc.dram_tensor("hTf", [HID, BL], F32, kind="ExternalInput").ap()
    d_bqk = nc.dram_tensor("bqk", [ATT, 1], F32, kind="ExternalInput").ap()
    d_bx = nc.dram_tensor("bx", [H3, 1], F32, kind="ExternalInput").ap()
    d_bhh = nc.dram_tensor("bhh", [H3, 1], F32, kind="ExternalInput").ap()

    d_h1 = nc.dram_tensor("h1", [BL, HID], F32, kind="ExternalOutput").ap()
    d_attn = nc.dram_tensor("attn", [BL, CTX], F32, kind="ExternalOutput").ap()

    with tile.TileContext(nc) as tc:
        _emit(tc, locals())
    nc.compile()
    return nc


def _emit(tc, d):
    from contextlib import ExitStack

    nc = tc.nc
    AT, CT, HT, H3T = ATT // 128, CTX // 128, HID // 128, H3 // 128  # 8,8,8,24
    ST = S // 128

    stack = ExitStack()
    pool = lambda *a, **k: stack.enter_context(tc.tile_pool(*a, **k))
    cst = pool(name="cst", bufs=1)
    actp = pool(name="actp", bufs=1)
    wkp = pool(name="wkp", bufs=1)
    wstream = pool(name="wstream", bufs=10)
    natp = pool(name="natp", bufs=16)
    natbp = pool(name="natbp", bufs=10)
    ctxTp = pool(name="ctxTp", bufs=12)
    tanhp = pool(name="tanhp", bufs=4)
    expp = pool(name="expp", bufs=2)
    rowp = pool(name="rowp", bufs=3)
    arowp = pool(name="arowp", bufs=2)
    ecolp = pool(name="ecolp", bufs=2)
    smallp = pool(name="smallp", bufs=6)
    gatep = pool(name="gatep", bufs=6)
    w2p = pool(name="w2p", bufs=3)

    # PSUM pools: 8 banks total (pc 4 + pl 2 + shared scratch 2)
    pc_ps = pool(name="pc_ps", bufs=4, space="PSUM")
    pl_ps = pool(name="pl_ps", bufs=2, space="PSUM")
    ms_ps = pool(name="ms_ps", bufs=2, space="PSUM")

    # ---- constants ----
    identb = cst.tile([128, 128], BF16, tag="identb")
    nc.sync.dma_start(identb[:], d["d_identb"][:])
    identf = cst.tile([128, 128], F32, tag="identf")
    nc.sync.dma_start(identf[:], d["d_identf"][:])
    ones1 = cst.tile([1, 128], F32R, tag="ones1")
    nc.sync.dma_start(ones1[:], d["d_ones1"][:])
    one1 = cst.tile([1, 1], F32, tag="one1")
    nc.sync.dma_start(one1[:], d["d_one1"][:])

    wl_sb = cst.tile([128, AT], F32R, tag="wl")
    nc.sync.dma_start(wl_sb[:], d["d_wlT"].rearrange("(t p) o -> p t o", p=128))
    bqk_sb = cst.tile([128, AT], F32, tag="bqk")
    nc.sync.dma_start(bqk_sb[:], d["d_bqk"].rearrange("(t p) o -> p t o", p=128))
    bx_sb = cst.tile([128, H3T], F32, tag="bx")
    bhh_sb = cst.tile([128, H3T], F32, tag="bhh")
    nc.sync.dma_start(bx_sb[:], d["d_bx"].rearrange("(t p) o -> p t o", p=128))
    nc.sync.dma_start(bhh_sb[:], d["d_bhh"].rearrange("(t p) o -> p t o", p=128))

    hT_sb = actp.tile([128, HT * BL], BF16, tag="hT")
    inT_sb = actp.tile([128, HT * BL], BF16, tag="inT")
    hTf_sb = actp.tile([128, HT * BL], F32, tag="hTf")
    nc.sync.dma_start(hT_sb[:], d["d_hT"].rearrange("(t p) b -> p t b", p=128))
    nc.sync.dma_start(inT_sb[:], d["d_inT"].rearrange("(t p) b -> p t b", p=128))
    nc.sync.dma_start(hTf_sb[:], d["d_hTf"].rearrange("(t p) b -> p t b", p=128))

    # Wk.T resident in bf16: block c -> wk_sb[:, c*ATT : (c+1)*ATT]
    wk_sb = wkp.tile([128, CT * ATT], BF16, tag="wk")
    for c in range(CT):
        nc.sync.dma_start(
            wk_sb[:, c * ATT : (c + 1) * ATT], d["d_wkT"][128 * c : 128 * (c + 1), :]
        )

    # ---- prologue: qeff = Wq@hiddenT + (bq+bk); gx1 = W1@inT + bx; gh = Whh@hT + bhh
    qeff = actp.tile([128, AT * BL], F32, tag="qeff")
    gx1 = actp.tile([128, H3T * BL], F32, tag="gx1")
    gh = actp.tile([128, H3T * BL], F32, tag="gh")
    TG = 2  # output tiles per weight DMA

    def emit_wgroup(dst, wname, bias_sb, rhs, t0):
        wt = wstream.tile([128, HT * TG * 128], BF16, tag="ws")
        nc.sync.dma_start(
            wt[:],
            d[wname][:, 128 * t0 : 128 * (t0 + TG)].rearrange(
                "(j p) m -> p j m", p=128
            ),
        )
        for tl in range(TG):
            t = t0 + tl
            pg = ms_ps.tile([128, BL], F32, tag="ms")
            for j in range(HT):
                lhs = wt[:, j * TG * 128 + tl * 128 : j * TG * 128 + (tl + 1) * 128]
                nc.tensor.matmul(
                    pg[:], lhs, rhs[:, j * BL : (j + 1) * BL],
                    start=(j == 0), stop=(j == HT - 1),
                )
            nc.scalar.activation(
                dst[:, t * BL : (t + 1) * BL], pg[:], AF.Identity,
                bias=bias_sb[:, t : t + 1],
            )

    for t0 in range(0, AT, TG):
        emit_wgroup(qeff, "d_wqT", bqk_sb, hT_sb, t0)
    # gx1/gh groups are interleaved into the batch loop (only needed at tail)
    wgroups = [("d_w1T", gx1, bx_sb, inT_sb, t0) for t0 in range(0, H3T, TG)]
    wgroups += [("d_whhT", gh, bhh_sb, hT_sb, t0) for t0 in range(0, H3T, TG)]

    # ---- main attention loop over local batch rows ----
    # Each batch row's softmax/attention tail is deferred and emitted at
    # checkpoints inside the NEXT row's cache-matmul loop so the PE's
    # in-order stream always has dense matmul work while DVE/ACT chase
    # the softmax dependency chain.
    sums = actp.tile([128, BL], F32, tag="sums")
    recip = actp.tile([128, BL], F32, tag="recip")
    attnT = actp.tile([128, CT * BL], BF16, tag="attnT")
    deferred = []  # closures carrying batch b-1's softmax/attn chunks

    def make_chunks(b, nat, lrow, prow):
        state = {}

        def run1():  # broadcast + max + exp
            pb0 = ms_ps.tile([128, 512], F32, tag="ms")
            pb1 = ms_ps.tile([128, 512], F32, tag="ms")
            for pb, sl in ((pb0, slice(0, 512)), (pb1, slice(512, 1024))):
                nc.tensor.matmul(pb[:], ones1[:], lrow[0:1, sl], start=True, stop=False)
                nc.tensor.matmul(pb[:], ones1[:], prow[0:1, sl], start=False, stop=True)
            mx2 = smallp.tile([128, 2], F32, tag="mx2")
            nc.vector.reduce_max(mx2[:, 0:1], pb0[:], axis=AX.X)
            nc.vector.reduce_max(mx2[:, 1:2], pb1[:], axis=AX.X)
            nmx = smallp.tile([128, 1], F32, tag="nmx")
            nc.vector.reduce_max(nmx[:], mx2[:], axis=AX.X, negate=True)
            acc2 = smallp.tile([128, 2], F32, tag="acc2")
            etile = expp.tile([128, S], F32, tag="exp")
            nc.scalar.activation(
                etile[:, 0:512], pb0[:], AF.Exp, bias=nmx[:], accum_out=acc2[:, 0:1]
            )
            nc.scalar.activation(
                etile[:, 512:1024], pb1[:], AF.Exp, bias=nmx[:], accum_out=acc2[:, 1:2]
            )
            nc.vector.tensor_add(sums[:, b : b + 1], acc2[:, 0:1], acc2[:, 1:2])
            nc.vector.reciprocal(recip[:, b : b + 1], sums[:, b : b + 1])
            state["etile"] = etile

        def run2():  # exp row -> column layout
            etile = state["etile"]
            pe = ms_ps.tile([128, ST], F32, tag="ms")
            for st in range(ST):
                nc.tensor.matmul(
                    pe[:, st : st + 1],
                    etile[0:1, 128 * st : 128 * (st + 1)],
                    one1[:],
                    start=True, stop=True,
                )
            ecol = ecolp.tile([128, ST], F32R, tag="ecol")
            nc.vector.tensor_copy(ecol[:], pe[:])
            state["ecol"] = ecol

        def run3():  # attention values + normalized output row
            ecol = state["ecol"]
            arow = arowp.tile([1, CTX], F32, tag="arow")
            for cg in range(2):
                pav = ms_ps.tile([1, 512], F32, tag="ms")
                for st in range(ST):
                    nc.tensor.matmul(
                        pav[:], ecol[:, st : st + 1],
                        _r(nat[st][:, 512 * cg : 512 * (cg + 1)]),
                        start=(st == 0), stop=(st == ST - 1),
                    )
                nc.vector.tensor_copy(arow[:, 512 * cg : 512 * (cg + 1)], pav[:])
            an = rowp.tile([1, CTX], F32, tag="row")
            nc.vector.tensor_scalar_mul(an[:], arow[:], recip[0:1, b : b + 1])
            nc.sync.dma_start(d["d_attn"][b : b + 1, :], an[:])
            state["arow"] = arow

        def run4():  # attnT columns for the W2 matmul
            arow = state["arow"]
            pat = ms_ps.tile([128, CT], F32, tag="ms")
            for c in range(CT):
                nc.tensor.matmul(
                    pat[:, c : c + 1],
                    arow[0:1, 128 * c : 128 * (c + 1)],
                    one1[:],
                    start=True, stop=True,
                )
            for c in range(CT):
                nc.vector.tensor_copy(
                    attnT[:, c * BL + b : c * BL + b + 1], pat[:, c : c + 1]
                )

        return [run1, run2, run3, run4]

    for b in range(BL):
        # natural context tiles [s_part, c_free] in f32r + bf16 copies
        nat, natb = [], []
        for st in range(ST):
            t = natp.tile([128, CTX], F32R, tag="nat")
            nc.scalar.dma_start(t[:], d["d_ctx"][b, 128 * st : 128 * (st + 1), :])
            nat.append(t)
            tb = natbp.tile([128, CTX], BF16, tag="natb")
            nc.gpsimd.dma_start(tb[:], t[:].bitcast(F32))
            natb.append(tb)

        # transpose to [c_part, s_free] via bf16 identity-matmuls
        ctxT = []
        for c in range(CT):
            tT = ctxTp.tile([128, S], BF16, tag="ctxT")
            for sg in range(2):
                pt = ms_ps.tile([128, 512], F32, tag="ms")
                for ss in range(4):
                    st = 4 * sg + ss
                    nc.tensor.matmul(
                        pt[:, 128 * ss : 128 * (ss + 1)],
                        natb[st][:, 128 * c : 128 * (c + 1)],
                        identb[:],
                        start=True, stop=True,
                    )
                nc.vector.tensor_copy(tT[:, 512 * sg : 512 * (sg + 1)], pt[:])
            ctxT.append(tT)

        # deferred prologue weight groups (4 per batch keeps DMA smooth)
        for _ in range(4):
            if wgroups:
                wn, dst_, bs_, rhs_, t0_ = wgroups.pop(0)
                emit_wgroup(dst_, wn, bs_, rhs_, t0_)

        # cache matmul + tanh + Wl reduction; pl matmuls for a-1 emitted
        # after the cache matmuls of a so the PE never waits on tanh.
        pl0 = pl_ps.tile([1, 512], F32, tag="pl")
        pl1 = pl_ps.tile([1, 512], F32, tag="pl")
        pending = None

        def emit_pl(th0, th1, a, pl0=pl0, pl1=pl1):
            nc.tensor.matmul(
                pl0[:], wl_sb[:, a : a + 1], _r(th0[:]),
                start=(a == 0), stop=(a == AT - 1),
            )
            nc.tensor.matmul(
                pl1[:], wl_sb[:, a : a + 1], _r(th1[:]),
                start=(a == 0), stop=(a == AT - 1),
            )

        for a in range(AT):
            pc0 = pc_ps.tile([128, 512], F32, tag="pc")
            pc1 = pc_ps.tile([128, 512], F32, tag="pc")
            for c in range(CT):
                lhs = wk_sb[:, c * ATT + 128 * a : c * ATT + 128 * (a + 1)]
                nc.tensor.matmul(
                    pc0[:], lhs, ctxT[c][:, 0:512],
                    start=(c == 0), stop=(c == CT - 1),
                )
                nc.tensor.matmul(
                    pc1[:], lhs, ctxT[c][:, 512:1024],
                    start=(c == 0), stop=(c == CT - 1),
                )
            if deferred and a in (0, 2, 4, 6):
                deferred[a // 2]()
            if pending is not None:
                emit_pl(*pending)
            th0 = tanhp.tile([128, 512], F32R, tag="tanh")
            th1 = tanhp.tile([128, 512], F32R, tag="tanh")
            qcol = qeff[:, a * BL + b : a * BL + b + 1]
            nc.scalar.activation(th0[:], pc0[:], AF.Tanh, bias=qcol)
            nc.scalar.activation(th1[:], pc1[:], AF.Tanh, bias=qcol)
            pending = (th0, th1, a)
        emit_pl(*pending)

        lrow = rowp.tile([1, S], F32R, tag="lrow")
        nc.vector.tensor_copy(lrow[:, 0:512], _r(pl0[:]))
        nc.vector.tensor_copy(lrow[:, 512:1024], _r(pl1[:]))
        prow = rowp.tile([1, S], F32R, tag="lrow")
        nc.sync.dma_start(prow[:], d["d_pen"][b : b + 1, :])

        deferred = make_chunks(b, nat, lrow, prow)

    for fn in deferred:  # flush last batch row
        fn()

    # ---- tail: gxa = W2 @ attnT (columns scaled by 1/sum); gates; hidden1
    r_all = actp.tile([128, HT * BL], F32, tag="r_all")
    z_all = actp.tile([128, HT * BL], F32, tag="z_all")
    h1nat = actp.tile([BL, HID], F32, tag="h1nat")
    for t0 in range(0, H3T, TG):
        wt2 = w2p.tile([128, CT * TG * 128], BF16, tag="w2s")
        nc.sync.dma_start(
            wt2[:],
            d["d_w2T"][:, 128 * t0 : 128 * (t0 + TG)].rearrange(
                "(j p) m -> p j m", p=128
            ),
        )
        for tl in range(TG):
            t = t0 + tl
            pg = ms_ps.tile([128, BL], F32, tag="ms")
            for c in range(CT):
                lhs = wt2[:, c * TG * 128 + tl * 128 : c * TG * 128 + (tl + 1) * 128]
                nc.tensor.matmul(
                    pg[:], lhs, attnT[:, c * BL : (c + 1) * BL],
                    start=(c == 0), stop=(c == CT - 1),
                )
            gxa = gatep.tile([128, BL], F32, tag="gxa")
            nc.vector.tensor_mul(gxa[:], pg[:], recip[:])
            gx = gatep.tile([128, BL], F32, tag="gx")
            nc.vector.tensor_add(gx[:], gxa[:], gx1[:, t * BL : (t + 1) * BL])
            ht = t % HT
            sl = slice(ht * BL, (ht + 1) * BL)
            if t < HT:  # r gate
                gs = gatep.tile([128, BL], F32, tag="gs")
                nc.vector.tensor_add(gs[:], gx[:], gh[:, t * BL : (t + 1) * BL])
                nc.scalar.activation(r_all[:, sl], gs[:], AF.Sigmoid)
            elif t < 2 * HT:  # z gate
                gs = gatep.tile([128, BL], F32, tag="gs")
                nc.vector.tensor_add(gs[:], gx[:], gh[:, t * BL : (t + 1) * BL])
                nc.scalar.activation(z_all[:, sl], gs[:], AF.Sigmoid)
            else:  # n gate, then hidden1 for this h-tile
                rh = gatep.tile([128, BL], F32, tag="rh")
                nc.vector.tensor_mul(rh[:], r_all[:, sl], gh[:, t * BL : (t + 1) * BL])
                ns = gatep.tile([128, BL], F32, tag="ns")
                nc.vector.tensor_add(ns[:], gx[:], rh[:])
                ntl = gatep.tile([128, BL], F32, tag="ntl")
                nc.scalar.activation(ntl[:], ns[:], AF.Tanh)
                hmn = gatep.tile([128, BL], F32, tag="hmn")
                nc.vector.tensor_sub(hmn[:], hTf_sb[:, sl], ntl[:])
                zh = gatep.tile([128, BL], F32, tag="zh")
                nc.vector.tensor_mul(zh[:], z_all[:, sl], hmn[:])
                h1T = gatep.tile([128, BL], F32, tag="h1T")
                nc.vector.tensor_add(h1T[:], ntl[:], zh[:])
                ph = ms_ps.tile([BL, 128], F32, tag="ms")
                nc.tensor.transpose(ph[:], h1T[:], identf[:])
                nc.vector.tensor_copy(h1nat[:, 128 * ht : 128 * (ht + 1)], ph[:])
    nc.sync.dma_start(d["d_h1"][:], h1nat[:])
    stack.close()


_NC_CACHE = None


def _get_program():
    global _NC_CACHE
    if _NC_CACHE is None:
        _NC_CACHE = build_program()
    return _NC_CACHE


def make_in_maps(inputs):
    """Host-side prep: shard batch across cores, transpose/fuse weights."""
    f = lambda x: np.ascontiguousarray(np.asarray(x, dtype=np.float32))
    bf = lambda x: np.ascontiguousarray(np.asarray(x, dtype=np.float32).astype(BF16NP))
    input_ = f(inputs["input"])
    hidden = f(inputs["hidden"])
    context = f(inputs["context"])
    mask = np.asarray(inputs["context_mask"])
    Wq, bq = f(inputs["Wq"]), f(inputs["bq"])
    Wk, bk = f(inputs["Wk"]), f(inputs["bk"])
    Wl = f(inputs["Wl"])
    We, be = f(inputs["We"]), f(inputs["be"])
    Wa, ba = f(inputs["Wa"]), f(inputs["ba"])
    W_ih, W_hh = f(inputs["W_ih"]), f(inputs["W_hh"])
    b_ih, b_hh = f(inputs["b_ih"]), f(inputs["b_hh"])

    shared = {
        "wkT": bf(Wk.T),
        "wqT": bf(Wq.T),
        "w1T": bf((W_ih @ We).T),
        "w2T": bf((W_ih @ Wa).T),
        "whhT": bf(W_hh.T),
        "wlT": np.ascontiguousarray(Wl.T),
        "bqk": np.ascontiguousarray((bq + bk).reshape(ATT, 1)),
        "bx": np.ascontiguousarray((W_ih @ (be + ba) + b_ih).reshape(H3, 1)),
        "bhh": np.ascontiguousarray(b_hh.reshape(H3, 1)),
        "identb": np.eye(128, dtype=BF16NP),
        "identf": np.eye(128, dtype=np.float32),
        "ones1": np.ones((1, 128), np.float32),
        "one1": np.ones((1, 1), np.float32),
    }
    pen = np.where(mask, np.float32(-1e18), np.float32(0.0)).astype(np.float32)
    inT = np.ascontiguousarray(input_.T)
    hT = np.ascontiguousarray(hidden.T)

    in_maps = []
    for k in range(NCORES):
        sl = slice(k * BL, (k + 1) * BL)
        in_maps.append(
            {
                "ctx": context[sl],
                "pen": np.ascontiguousarray(pen[sl]),
                "inT": np.ascontiguousarray(inT[:, sl]).astype(BF16NP),
                "hT": np.ascontiguousarray(hT[:, sl]).astype(BF16NP),
                "hTf": np.ascontiguousarray(hT[:, sl]),
                **shared,
            }
        )
    return in_maps


def kernel(**inputs):
    nc = _get_program()
    in_maps = make_in_maps(inputs)
    res = run_bass_kernel_spmd(nc, in_maps, core_ids=list(range(NCORES)))
    hidden1 = np.concatenate([res.results[k]["h1"] for k in range(NCORES)], axis=0)
    attn = np.concatenate([res.results[k]["attn"] for k in range(NCORES)], axis=0)
    return (hidden1, attn)


# revision 22
# speedup vs baseline: 1.9693x; 1.0818x over previous
"""Trainium2 Bass kernel for an attention-augmented GRU cell (CGRUCell).

Reference computation (per batch row):
    cache   = context @ Wk.T + bk                  # [S, A]
    q       = hidden @ Wq.T + bq                   # [A]
    logits  = tanh(q + cache) @ Wl[0] + bl         # [S]
    logits  = where(mask, -1e18, logits)
    w       = softmax(logits)                      # [S]
    attn    = w @ context                          # [CTX]
    x       = input @ We.T + be + attn @ Wa.T + ba
    gx      = x @ W_ih.T + b_ih ; gh = hidden @ W_hh.T + b_hh
    r, z    = sigmoid(gx_r + gh_r), sigmoid(gx_z + gh_z)
    n       = tanh(gx_n + r * gh_n)
    hidden1 = (1 - z) * n + z * hidden
Outputs: (hidden1, attn)

Strategy: data-parallel over batch on 8 NeuronCores (8 rows each). The
dominant work is the [S,CTX]@[CTX,A] key projection; it runs on the
TensorEngine in bf16 (context is cast on the otherwise-idle GpSimd
engine, then transposed with cheap bf16 identity-matmuls so the
contraction dim lands on partitions). The softmax reduction over the
attention dim rides on f32r matmuls against Wl with the q/bk bias fused
into the tanh Activation op; the attention-value matvec contracts the
full-precision f32r context so the attn output keeps ~1e-4 accuracy.
The GRU algebra is reassociated (W1 = W_ih@We, W2 = W_ih@Wa) so all of
it except attn @ W2.T is computed in a prologue from the raw inputs.
The softmax/attention tail of each batch row is emitted interleaved
into the next row's cache matmuls so the in-order PE never idles on
DVE/ACT latency.
"""

import sys

if "/opt/trn_rl_repo" not in sys.path:
    sys.path.insert(0, "/opt/trn_rl_repo")

import ml_dtypes
import numpy as np

import concourse.bass as bass
import concourse.tile as tile
from concourse import bacc, mybir
from concourse.bass_utils import run_bass_kernel_spmd

NCORES = 8
B, S, IN, HID, CTX, ATT = 64, 1024, 1024, 1024, 1024, 1024
BL = B // NCORES          # batch rows per core
H3 = 3 * HID
F32 = mybir.dt.float32
F32R = mybir.dt.float32r
BF16 = mybir.dt.bfloat16
AX = mybir.AxisListType
AF = mybir.ActivationFunctionType
BF16NP = ml_dtypes.bfloat16


def _r(ap):
    return ap.bitcast(F32R)


def build_program():
    nc = bacc.Bacc("TRN2", target_bir_lowering=False, debug=False, num_devices=NCORES)

    d_ctx = nc.dram_tensor("ctx", [BL, S, CTX], F32R, kind="ExternalInput").ap()
    d_pen = nc.dram_tensor("pen", [BL, S], F32R, kind="ExternalInput").ap()
    d_wlrep = nc.dram_tensor("wlrep", [ATT, 128], F32R, kind="ExternalInput").ap()
    d_ones1 = nc.dram_tensor("ones1", [1, 128], F32R, kind="ExternalInput").ap()
    d_wkT = nc.dram_tensor("wkT", [CTX, ATT], BF16, kind="ExternalInput").ap()
    d_wqT = nc.dram_tensor("wqT", [HID, ATT], BF16, kind="ExternalInput").ap()
    d_w1T = nc.dram_tensor("w1T", [IN, H3], BF16, kind="ExternalInput").ap()
    d_w2T = nc.dram_tensor("w2T", [CTX, H3], BF16, kind="ExternalInput").ap()
    d_whhT = nc.dram_tensor("whhT", [HID, H3], BF16, kind="ExternalInput").ap()
    d_hT = nc.dram_tensor("hT", [HID, BL], BF16, kind="ExternalInput").ap()
    d_inT = nc.dram_tensor("inT", [IN, BL], BF16, kind="ExternalInput").ap()
    d_identb = nc.dram_tensor("identb", [128, 128], BF16, kind="ExternalInput").ap()
    d_identf = nc.dram_tensor("identf", [128, 128], F32, kind="ExternalInput").ap()
    d_one1 = nc.dram_tensor("one1", [1, 1], F32, kind="ExternalInput").ap()
    d_hTf = nc.dram_tensor("hTf", [HID, BL], F32, kind="ExternalInput").ap()
    d_bqk = nc.dram_tensor("bqk", [ATT, 1], F32, kind="ExternalInput").ap()
    d_bx = nc.dram_tensor("bx", [H3, 1], F32, kind="ExternalInput").ap()
    d_bhh = nc.dram_tensor("bhh", [H3, 1], F32, kind="ExternalInput").ap()

    d_h1 = nc.dram_tensor("h1", [BL, HID], F32, kind="ExternalOutput").ap()
    d_attn = nc.dram_tensor("attn", [BL, CTX], F32, kind="ExternalOutput").ap()

    with tile.TileContext(nc) as tc:
        _emit(tc, locals())
    nc.compile()
    return nc


def _emit(tc, d):
    from contextlib import ExitStack

    nc = tc.nc
    AT, CT, HT, H3T = ATT // 128, CTX // 128, HID // 128, H3 // 128  # 8,8,8,24
    ST = S // 128

    stack = ExitStack()
    pool = lambda *a, **k: stack.enter_context(tc.tile_pool(*a, **k))
    cst = pool(name="cst", bufs=1)
    actp = pool(name="actp", bufs=1)
    wkp = pool(name="wkp", bufs=1)
    wstream = pool(name="wstream", bufs=10)
    natp = pool(name="natp", bufs=16)
    natbp = pool(name="natbp", bufs=10)
    ctxTp = pool(name="ctxTp", bufs=12)
    tanhp = pool(name="tanhp", bufs=4)
    expp = pool(name="expp", bufs=2)
    rowp = pool(name="rowp", bufs=3)
    arowp = pool(name="arowp", bufs=2)
    ecolp = pool(name="ecolp", bufs=2)
    smallp = pool(name="smallp", bufs=6)
    gatep = pool(name="gatep", bufs=6)
    w2p = pool(name="w2p", bufs=3)

    # PSUM pools: 8 banks total (pc 4 + pl 2 + shared scratch 2)
    pc_ps = pool(name="pc_ps", bufs=4, space="PSUM")
    pl_ps = pool(name="pl_ps", bufs=2, space="PSUM")
    ms_ps = pool(name="ms_ps", bufs=2, space="PSUM")

    # ---- constants ----
    identb = cst.tile([128, 128], BF16, tag="identb")
    nc.sync.dma_start(identb[:], d["d_identb"][:])
    identf = cst.tile([128, 128], F32, tag="identf")
    nc.sync.dma_start(identf[:], d["d_identf"][:])
    ones1 = cst.tile([1, 128], F32R, tag="ones1")
    nc.sync.dma_start(ones1[:], d["d_ones1"][:])
    one1 = cst.tile([1, 1], F32, tag="one1")
    nc.sync.dma_start(one1[:], d["d_one1"][:])

    wlrep_sb = cst.tile([128, AT * 128], F32R, tag="wlrep")
    nc.sync.dma_start(
        wlrep_sb[:], d["d_wlrep"].rearrange("(t p) c -> p t c", p=128)
    )
    bqk_sb = cst.tile([128, AT], F32, tag="bqk")
    nc.sync.dma_start(bqk_sb[:], d["d_bqk"].rearrange("(t p) o -> p t o", p=128))
    bx_sb = cst.tile([128, H3T], F32, tag="bx")
    bhh_sb = cst.tile([128, H3T], F32, tag="bhh")
    nc.sync.dma_start(bx_sb[:], d["d_bx"].rearrange("(t p) o -> p t o", p=128))
    nc.sync.dma_start(bhh_sb[:], d["d_bhh"].rearrange("(t p) o -> p t o", p=128))

    hT_sb = actp.tile([128, HT * BL], BF16, tag="hT")
    inT_sb = actp.tile([128, HT * BL], BF16, tag="inT")
    hTf_sb = actp.tile([128, HT * BL], F32, tag="hTf")
    nc.sync.dma_start(hT_sb[:], d["d_hT"].rearrange("(t p) b -> p t b", p=128))
    nc.sync.dma_start(inT_sb[:], d["d_inT"].rearrange("(t p) b -> p t b", p=128))
    nc.sync.dma_start(hTf_sb[:], d["d_hTf"].rearrange("(t p) b -> p t b", p=128))

    # Wk.T resident in bf16: block c -> wk_sb[:, c*ATT : (c+1)*ATT]
    wk_sb = wkp.tile([128, CT * ATT], BF16, tag="wk")
    for c in range(CT):
        nc.sync.dma_start(
            wk_sb[:, c * ATT : (c + 1) * ATT], d["d_wkT"][128 * c : 128 * (c + 1), :]
        )

    # ---- prologue: qeff = Wq@hiddenT + (bq+bk); gx1 = W1@inT + bx; gh = Whh@hT + bhh
    qeff = actp.tile([128, AT * BL], F32, tag="qeff")
    gx1 = actp.tile([128, H3T * BL], F32, tag="gx1")
    gh = actp.tile([128, H3T * BL], F32, tag="gh")
    TG = 2  # output tiles per weight DMA

    def emit_wgroup(dst, wname, bias_sb, rhs, t0):
        wt = wstream.tile([128, HT * TG * 128], BF16, tag="ws")
        nc.sync.dma_start(
            wt[:],
            d[wname][:, 128 * t0 : 128 * (t0 + TG)].rearrange(
                "(j p) m -> p j m", p=128
            ),
        )
        for tl in range(TG):
            t = t0 + tl
            pg = ms_ps.tile([128, BL], F32, tag="ms")
            for j in range(HT):
                lhs = wt[:, j * TG * 128 + tl * 128 : j * TG * 128 + (tl + 1) * 128]
                nc.tensor.matmul(
                    pg[:], lhs, rhs[:, j * BL : (j + 1) * BL],
                    start=(j == 0), stop=(j == HT - 1),
                )
            nc.scalar.activation(
                dst[:, t * BL : (t + 1) * BL], pg[:], AF.Identity,
                bias=bias_sb[:, t : t + 1],
            )

    for t0 in range(0, AT, TG):
        emit_wgroup(qeff, "d_wqT", bqk_sb, hT_sb, t0)
    # gx1/gh groups are interleaved into the batch loop (only needed at tail)
    wgroups = [("d_w1T", gx1, bx_sb, inT_sb, t0) for t0 in range(0, H3T, TG)]
    wgroups += [("d_whhT", gh, bhh_sb, hT_sb, t0) for t0 in range(0, H3T, TG)]

    # ---- main attention loop over local batch rows ----
    # Batch row b's softmax/attention tail is deferred and emitted at
    # checkpoints inside row b+1's cache-matmul loop so the in-order PE
    # always has dense matmul work while DVE/ACT chase the softmax
    # dependency chain. Context loads for b+1 are issued from a
    # checkpoint inside b's cache loop so the transposes never wait.
    sums = actp.tile([128, BL], F32, tag="sums")
    recip = actp.tile([128, BL], F32, tag="recip")
    attnT = actp.tile([128, CT * BL], BF16, tag="attnT")
    deferred = []  # closures carrying batch b-1's softmax/attn chunks
    loads = {}

    def preload(b):
        nat, natb = [], []
        for st in range(ST):
            t = natp.tile([128, CTX], F32R, tag="nat")
            nc.scalar.dma_start(t[:], d["d_ctx"][b, 128 * st : 128 * (st + 1), :])
            nat.append(t)
            tb = natbp.tile([128, CTX], BF16, tag="natb")
            nc.gpsimd.dma_start(tb[:], t[:].bitcast(F32))
            natb.append(tb)
        loads[b] = (nat, natb)

    def make_chunks(b, nat, plb0, plb1):
        state = {}

        def run1():  # max + exp straight off the broadcast-logits psum
            mx2 = smallp.tile([128, 2], F32, tag="mx2")
            nc.vector.reduce_max(mx2[:, 0:1], plb0[:], axis=AX.X)
            nc.vector.reduce_max(mx2[:, 1:2], plb1[:], axis=AX.X)
            nmx = smallp.tile([128, 1], F32, tag="nmx")
            nc.vector.reduce_max(nmx[:], mx2[:], axis=AX.X, negate=True)
            acc2 = smallp.tile([128, 2], F32, tag="acc2")
            etile = expp.tile([128, S], F32, tag="exp")
            nc.scalar.activation(
                etile[:, 0:512], plb0[:], AF.Exp, bias=nmx[:], accum_out=acc2[:, 0:1]
            )
            nc.scalar.activation(
                etile[:, 512:1024], plb1[:], AF.Exp, bias=nmx[:], accum_out=acc2[:, 1:2]
            )
            nc.vector.tensor_add(sums[:, b : b + 1], acc2[:, 0:1], acc2[:, 1:2])
            nc.vector.reciprocal(recip[:, b : b + 1], sums[:, b : b + 1])
            state["etile"] = etile

        def run2():  # exp row -> column layout
            etile = state["etile"]
            pe = ms_ps.tile([128, ST], F32, tag="ms")
            for st in range(ST):
                nc.tensor.matmul(
                    pe[:, st : st + 1],
                    etile[0:1, 128 * st : 128 * (st + 1)],
                    one1[:],
                    start=True, stop=True,
                )
            ecol = ecolp.tile([128, ST], F32R, tag="ecol")
            nc.vector.tensor_copy(ecol[:], pe[:])
            state["ecol"] = ecol

        def run3():  # attention values + normalized output row
            ecol = state["ecol"]
            arow = arowp.tile([1, CTX], F32, tag="arow")
            for cg in range(2):
                pav = ms_ps.tile([1, 512], F32, tag="ms")
                for st in range(ST):
                    nc.tensor.matmul(
                        pav[:], ecol[:, st : st + 1],
                        _r(nat[st][:, 512 * cg : 512 * (cg + 1)]),
                        start=(st == 0), stop=(st == ST - 1),
                    )
                nc.vector.tensor_copy(arow[:, 512 * cg : 512 * (cg + 1)], pav[:])
            an = arowp.tile([1, CTX], F32, tag="arow")
            nc.vector.tensor_scalar_mul(an[:], arow[:], recip[0:1, b : b + 1])
            nc.sync.dma_start(d["d_attn"][b : b + 1, :], an[:])
            state["arow"] = arow

        def run4():  # attnT columns for the W2 matmul
            arow = state["arow"]
            pat = ms_ps.tile([128, CT], F32, tag="ms")
            for c in range(CT):
                nc.tensor.matmul(
                    pat[:, c : c + 1],
                    arow[0:1, 128 * c : 128 * (c + 1)],
                    one1[:],
                    start=True, stop=True,
                )
            for c in range(CT):
                nc.vector.tensor_copy(
                    attnT[:, c * BL + b : c * BL + b + 1], pat[:, c : c + 1]
                )

        return [run1, run2, run3, run4]

    preload(0)
    for b in range(BL):
        nat, natb = loads.pop(b)
        prow = rowp.tile([1, S], F32R, tag="lrow")
        nc.sync.dma_start(prow[:], d["d_pen"][b : b + 1, :])

        if deferred:
            deferred[0]()  # DVE/ACT only: frees the pl psum banks early

        # transpose to [c_part, s_free] via bf16 identity-matmuls
        ctxT = []
        for c in range(CT):
            tT = ctxTp.tile([128, S], BF16, tag="ctxT")
            for sg in range(2):
                pt = ms_ps.tile([128, 512], F32, tag="ms")
                for ss in range(4):
                    st = 4 * sg + ss
                    nc.tensor.matmul(
                        pt[:, 128 * ss : 128 * (ss + 1)],
                        natb[st][:, 128 * c : 128 * (c + 1)],
                        identb[:],
                        start=True, stop=True,
                    )
                nc.vector.tensor_copy(tT[:, 512 * sg : 512 * (sg + 1)], pt[:])
            ctxT.append(tT)

        # deferred prologue weight groups (keeps their DMA off the start)
        for _ in range(4):
            if wgroups:
                wn, dst_, bs_, rhs_, t0_ = wgroups.pop(0)
                emit_wgroup(dst_, wn, bs_, rhs_, t0_)

        # cache matmul + tanh + broadcast-logits reduction. The replicated
        # Wl stationary operand makes the Wl-contraction emit logits
        # replicated across all 128 partitions, ready for softmax. pl
        # matmuls for a-1 are emitted after the cache matmuls of a so the
        # PE never waits on the tanh ACT drain.
        plb0 = pl_ps.tile([128, 512], F32, tag="pl")
        plb1 = pl_ps.tile([128, 512], F32, tag="pl")
        pending = None

        def emit_pl(th0, th1, a, plb0=plb0, plb1=plb1):
            lhs = _r(wlrep_sb[:, a * 128 : (a + 1) * 128])
            nc.tensor.matmul(plb0[:], lhs, _r(th0[:]), start=(a == 0), stop=False)
            nc.tensor.matmul(plb1[:], lhs, _r(th1[:]), start=(a == 0), stop=False)

        for a in range(AT):
            pc0 = pc_ps.tile([128, 512], F32, tag="pc")
            pc1 = pc_ps.tile([128, 512], F32, tag="pc")
            for c in range(CT):
                lhs = wk_sb[:, c * ATT + 128 * a : c * ATT + 128 * (a + 1)]
                nc.tensor.matmul(
                    pc0[:], lhs, ctxT[c][:, 0:512],
                    start=(c == 0), stop=(c == CT - 1),
                )
                nc.tensor.matmul(
                    pc1[:], lhs, ctxT[c][:, 512:1024],
                    start=(c == 0), stop=(c == CT - 1),
                )
            if deferred and a in (0, 2, 4):
                deferred[a // 2 + 1]()
            if a == 5 and b + 1 < BL:
                preload(b + 1)
            if pending is not None:
                emit_pl(*pending)
            th0 = tanhp.tile([128, 512], F32R, tag="tanh")
            th1 = tanhp.tile([128, 512], F32R, tag="tanh")
            qcol = qeff[:, a * BL + b : a * BL + b + 1]
            nc.scalar.activation(th0[:], pc0[:], AF.Tanh, bias=qcol)
            nc.scalar.activation(th1[:], pc1[:], AF.Tanh, bias=qcol)
            pending = (th0, th1, a)
        emit_pl(*pending)
        # fold the additive mask penalties into the broadcast logits
        nc.tensor.matmul(plb0[:], ones1[:], prow[0:1, 0:512], start=False, stop=True)
        nc.tensor.matmul(plb1[:], ones1[:], prow[0:1, 512:1024], start=False, stop=True)

        deferred = make_chunks(b, nat, plb0, plb1)

    for fn in deferred:  # flush last batch row
        fn()

    # ---- tail: gxa = W2 @ attnT (columns scaled by 1/sum); gates; hidden1
    r_all = actp.tile([128, HT * BL], F32, tag="r_all")
    z_all = actp.tile([128, HT * BL], F32, tag="z_all")
    h1nat = actp.tile([BL, HID], F32, tag="h1nat")
    for t0 in range(0, H3T, TG):
        wt2 = w2p.tile([128, CT * TG * 128], BF16, tag="w2s")
        nc.sync.dma_start(
            wt2[:],
            d["d_w2T"][:, 128 * t0 : 128 * (t0 + TG)].rearrange(
                "(j p) m -> p j m", p=128
            ),
        )
        for tl in range(TG):
            t = t0 + tl
            pg = ms_ps.tile([128, BL], F32, tag="ms")
            for c in range(CT):
                lhs = wt2[:, c * TG * 128 + tl * 128 : c * TG * 128 + (tl + 1) * 128]
                nc.tensor.matmul(
                    pg[:], lhs, attnT[:, c * BL : (c + 1) * BL],
                    start=(c == 0), stop=(c == CT - 1),
                )
            gxa = gatep.tile([128, BL], F32, tag="gxa")
            nc.vector.tensor_mul(gxa[:], pg[:], recip[:])
            gx = gatep.tile([128, BL], F32, tag="gx")
            nc.vector.tensor_add(gx[:], gxa[:], gx1[:, t * BL : (t + 1) * BL])
            ht = t % HT
            sl = slice(ht * BL, (ht + 1) * BL)
            if t < HT:  # r gate
                gs = gatep.tile([128, BL], F32, tag="gs")
                nc.vector.tensor_add(gs[:], gx[:], gh[:, t * BL : (t + 1) * BL])
                nc.scalar.activation(r_all[:, sl], gs[:], AF.Sigmoid)
            elif t < 2 * HT:  # z gate
                gs = gatep.tile([128, BL], F32, tag="gs")
                nc.vector.tensor_add(gs[:], gx[:], gh[:, t * BL : (t + 1) * BL])
                nc.scalar.activation(z_all[:, sl], gs[:], AF.Sigmoid)
            else:  # n gate, then hidden1 for this h-tile
                rh = gatep.tile([128, BL], F32, tag="rh")
                nc.vector.tensor_mul(rh[:], r_all[:, sl], gh[:, t * BL : (t + 1) * BL])
                ns = gatep.tile([128, BL], F32, tag="ns")
                nc.vector.tensor_add(ns[:], gx[:], rh[:])
                ntl = gatep.tile([128, BL], F32, tag="ntl")
                nc.scalar.activation(ntl[:], ns[:], AF.Tanh)
                hmn = gatep.tile([128, BL], F32, tag="hmn")
                nc.vector.tensor_sub(hmn[:], hTf_sb[:, sl], ntl[:])
                zh = gatep.tile([128, BL], F32, tag="zh")
                nc.vector.tensor_mul(zh[:], z_all[:, sl], hmn[:])
                h1T = gatep.tile([128, BL], F32, tag="h1T")
                nc.vector.tensor_add(h1T[:], ntl[:], zh[:])
                ph = ms_ps.tile([BL, 128], F32, tag="ms")
                nc.tensor.transpose(ph[:], h1T[:], identf[:])
                nc.vector.tensor_copy(h1nat[:, 128 * ht : 128 * (ht + 1)], ph[:])
    nc.sync.dma_start(d["d_h1"][:], h1nat[:])
    stack.close()


_NC_CACHE = None


def _get_program():
    global _NC_CACHE
    if _NC_CACHE is None:
        _NC_CACHE = build_program()
    return _NC_CACHE


def make_in_maps(inputs):
    """Host-side prep: shard batch across cores, transpose/fuse weights."""
    f = lambda x: np.ascontiguousarray(np.asarray(x, dtype=np.float32))
    bf = lambda x: np.ascontiguousarray(np.asarray(x, dtype=np.float32).astype(BF16NP))
    input_ = f(inputs["input"])
    hidden = f(inputs["hidden"])
    context = f(inputs["context"])
    mask = np.asarray(inputs["context_mask"])
    Wq, bq = f(inputs["Wq"]), f(inputs["bq"])
    Wk, bk = f(inputs["Wk"]), f(inputs["bk"])
    Wl = f(inputs["Wl"])
    We, be = f(inputs["We"]), f(inputs["be"])
    Wa, ba = f(inputs["Wa"]), f(inputs["ba"])
    W_ih, W_hh = f(inputs["W_ih"]), f(inputs["W_hh"])
    b_ih, b_hh = f(inputs["b_ih"]), f(inputs["b_hh"])

    shared = {
        "wkT": bf(Wk.T),
        "wqT": bf(Wq.T),
        "w1T": bf((W_ih @ We).T),
        "w2T": bf((W_ih @ Wa).T),
        "whhT": bf(W_hh.T),
        "wlrep": np.ascontiguousarray(np.tile(Wl.T, (1, 128))),
        "bqk": np.ascontiguousarray((bq + bk).reshape(ATT, 1)),
        "bx": np.ascontiguousarray((W_ih @ (be + ba) + b_ih).reshape(H3, 1)),
        "bhh": np.ascontiguousarray(b_hh.reshape(H3, 1)),
        "identb": np.eye(128, dtype=BF16NP),
        "identf": np.eye(128, dtype=np.float32),
        "ones1": np.ones((1, 128), np.float32),
        "one1": np.ones((1, 1), np.float32),
    }
    pen = np.where(mask, np.float32(-1e18), np.float32(0.0)).astype(np.float32)
    inT = np.ascontiguousarray(input_.T)
    hT = np.ascontiguousarray(hidden.T)

    in_maps = []
    for k in range(NCORES):
        sl = slice(k * BL, (k + 1) * BL)
        in_maps.append(
            {
                "ctx": context[sl],
                "pen": np.ascontiguousarray(pen[sl]),
                "inT": np.ascontiguousarray(inT[:, sl]).astype(BF16NP),
                "hT": np.ascontiguousarray(hT[:, sl]).astype(BF16NP),
                "hTf": np.ascontiguousarray(hT[:, sl]),
                **shared,
            }
        )
    return in_maps


def kernel(**inputs):
    nc = _get_program()
    in_maps = make_in_maps(inputs)
    res = run_bass_kernel_spmd(nc, in_maps, core_ids=list(range(NCORES)))
    hidden1 = np.concatenate([res.results[k]["h1"] for k in range(NCORES)], axis=0)
    attn = np.concatenate([res.results[k]["attn"] for k in range(NCORES)], axis=0)
    return (hidden1, attn)
